# revision 1
# baseline (speedup 1.0000x reference)
"""MoE (8 experts, top-2) expert-parallel Trainium2 kernel.

Contract: kernel(**inputs) takes the full unsharded inputs and returns the
full [8, 2048, 768] output.  Internally:
  - host computes the gate (scores -> top-2 -> softmax) in float64 and
    dispatches tokens to experts (the "all-to-all" of the sharding hint),
  - each of the 8 NeuronCores runs one expert's 3-layer GELU MLP over its
    routed tokens (padded to a common capacity C) via a Bass/Tile kernel,
  - host combines expert outputs with the gate weights.

Device math is float32r (TF32-class matmul inputs, fp32 PSUM accumulate),
~1.5e-4 relative error per layer vs fp32.
"""

import os
import sys
import types

import numpy as np

import concourse.bass as bass  # noqa: F401  (bass must import before mybir use)
import concourse.mybir as mybir
from concourse import bacc
from concourse.tile import TileContext
from concourse.bass_utils import run_bass_kernel_spmd

EMB, HID, HID2 = 768, 3072, 6144
NE, TOPK = 8, 2
P = 128  # partitions


def _install_ntff_hook():
    """Make trace=True work when antenv.axon_hooks is missing in the image."""
    try:
        from antenv.axon_hooks import get_axon_ntff_profile_hook  # noqa: F401
        return
    except ImportError:
        pass
    try:
        from trn_agent_boot.trn_boot import _ntff_profile_via_ctypes
        hook = _ntff_profile_via_ctypes('/opt/axon/libaxon_pjrt.so')
        mod = types.ModuleType('antenv.axon_hooks')
        mod.get_axon_ntff_profile_hook = lambda: hook
        sys.modules['antenv.axon_hooks'] = mod
    except Exception:
        pass


def _nsub_splits(length, max_n=512, min_n=256):
    """Split `length` into pieces <= max_n, each >= min_n (for full-rate
    f32r matmuls).  length >= min_n assumed unless length < min_n."""
    if length <= max_n:
        return [length]
    pieces = []
    rem = length
    while rem > max_n + min_n:
        pieces.append(max_n)
        rem -= max_n
    if rem <= max_n:
        pieces.append(rem)
    else:
        a = rem // 2
        pieces.extend([rem - a, a])
    return pieces


def _chunk_splits(c_tiles, max_tiles=12):
    """Split C (in 128-token tiles) into chunks of <= max_tiles tiles."""
    n_chunks = -(-c_tiles // max_tiles)
    base, extra = divmod(c_tiles, n_chunks)
    return [(base + (1 if i < extra else 0)) * P for i in range(n_chunks)]


def _build_program(C):
    """Build the per-core SPMD Bass program for capacity-C tokens."""
    f32 = mybir.dt.float32
    f32r = mybir.dt.float32r
    GELU = mybir.ActivationFunctionType.Gelu
    IDENT = mybir.ActivationFunctionType.Identity

    K1, K2, K3 = EMB // P, HID // P, HID2 // P          # 6, 24, 48 k-tiles
    MB1, MB2 = HID // 256, HID2 // 256                  # 12, 24 m-blocks (256 wide)

    nc = bacc.Bacc(None, target_bir_lowering=False)

    XT = nc.declare_dram_parameter("XT", [K1, P, C], f32r, isOutput=False)
    W1B = nc.declare_dram_parameter("W1B", [MB1, K1, P, 256], f32r, isOutput=False)
    W2B = nc.declare_dram_parameter("W2B", [MB2, K2, P, 256], f32r, isOutput=False)
    W3P = nc.declare_dram_parameter("W3P", [K3, P, EMB], f32r, isOutput=False)
    B1 = nc.declare_dram_parameter("B1", [P, K2], f32, isOutput=False)
    B2 = nc.declare_dram_parameter("B2", [P, K3], f32, isOutput=False)
    B3 = nc.declare_dram_parameter("B3", [P, K1], f32, isOutput=False)
    YT = nc.declare_dram_parameter("YT", [K1, P, C], f32, isOutput=True)
    H2D = nc.dram_tensor("H2D", [K3, P, C], f32r)       # layer-2 spill

    chunks = _chunk_splits(C // P)

    with TileContext(nc) as tc:
        with (
            tc.tile_pool(name="bias", bufs=1) as bias_pool,
            tc.tile_pool(name="xt", bufs=1) as xt_pool,
            tc.tile_pool(name="h1", bufs=1) as h1_pool,
            tc.tile_pool(name="wst", bufs=24) as w_pool,
            tc.tile_pool(name="ev", bufs=8) as ev_pool,
            tc.tile_pool(name="psA", bufs=8, space="PSUM") as ps_pool,
        ):
            b1t = bias_pool.tile([P, K2], f32)
            b2t = bias_pool.tile([P, K3], f32)
            nc.sync.dma_start(b1t[:], B1[:])
            nc.sync.dma_start(b2t[:], B2[:])

            c0 = 0
            for ci, tc_len in enumerate(chunks):
                subs = _nsub_splits(tc_len)
                offs = [sum(subs[:i]) for i in range(len(subs))]
                xts = []
                for k in range(K1):
                    t = xt_pool.tile([P, tc_len], f32r, tag=f"xt{k}", name=f"xt{ci}_{k}")
                    nc.sync.dma_start(t[:], XT[k, :, c0:c0 + tc_len])
                    xts.append(t)
                h1s = []
                for k in range(K2):
                    t = h1_pool.tile([P, tc_len], f32r, tag=f"h1_{k}", name=f"h1_{ci}_{k}")
                    h1s.append(t)

                # ---- layer 1: H1 = gelu(X @ W1 + b1), feature-major ----
                for mb in range(MB1):
                    ps = {}
                    for j in range(2):
                        for s in range(len(subs)):
                            ps[j, s] = ps_pool.tile([P, subs[s]], f32, tag="ps",
                                                    name=f"l1ps{ci}_{mb}_{j}_{s}")
                    for k in range(K1):
                        wt = w_pool.tile([P, 256], f32r, tag="w", name=f"w1_{ci}_{mb}_{k}")
                        nc.sync.dma_start(wt[:], W1B[mb, k])
                        for j in range(2):
                            for s, (o, ln) in enumerate(zip(offs, subs)):
                                nc.tensor.matmul(
                                    ps[j, s][:], wt[:, j * P:(j + 1) * P],
                                    xts[k][:, o:o + ln],
                                    start=(k == 0), stop=(k == K1 - 1))
                    for j in range(2):
                        jj = 2 * mb + j
                        for s, (o, ln) in enumerate(zip(offs, subs)):
                            nc.scalar.activation(h1s[jj][:, o:o + ln], ps[j, s][:],
                                                 GELU, bias=b1t[:, jj:jj + 1])

                # ---- layer 2: H2 = gelu(H1 @ W2 + b2) -> DRAM spill ----
                for mb in range(MB2):
                    ps = {}
                    for j in range(2):
                        for s in range(len(subs)):
                            ps[j, s] = ps_pool.tile([P, subs[s]], f32, tag="ps",
                                                    name=f"l2ps{ci}_{mb}_{j}_{s}")
                    for k in range(K2):
                        wt = w_pool.tile([P, 256], f32r, tag="w", name=f"w2_{ci}_{mb}_{k}")
                        nc.sync.dma_start(wt[:], W2B[mb, k])
                        for j in range(2):
                            for s, (o, ln) in enumerate(zip(offs, subs)):
                                nc.tensor.matmul(
                                    ps[j, s][:], wt[:, j * P:(j + 1) * P],
                                    h1s[k][:, o:o + ln],
                                    start=(k == 0), stop=(k == K2 - 1))
                    for j in range(2):
                        jj = 2 * mb + j
                        for s, (o, ln) in enumerate(zip(offs, subs)):
                            ev = ev_pool.tile([P, 512], f32r, tag="ev")
                            nc.scalar.activation(ev[:, :ln], ps[j, s][:],
                                                 GELU, bias=b2t[:, jj:jj + 1])
                            nc.sync.dma_start(H2D[jj, :, c0 + o:c0 + o + ln], ev[:, :ln])
                c0 += tc_len

        # ---- layer 3: Y = H2 @ W3 + b3, W3 resident ----
        with (
            tc.tile_pool(name="bias3", bufs=1) as b3_pool,
            tc.tile_pool(name="w3", bufs=1) as w3_pool,
            tc.tile_pool(name="h2s", bufs=10) as h2_pool,
            tc.tile_pool(name="yev", bufs=8) as y_pool,
            tc.tile_pool(name="psB", bufs=8, space="PSUM") as psB_pool,
        ):
            b3t = b3_pool.tile([P, K1], f32)
            nc.sync.dma_start(b3t[:], B3[:])
            w3s = []
            for k in range(K3):
                t = w3_pool.tile([P, EMB], f32r, tag=f"w3_{k}")
                nc.sync.dma_start(t[:], W3P[k])
                w3s.append(t)
            subsB = []
            o = 0
            for ln in _nsub_splits(C):
                subsB.append((o, ln))
                o += ln
            for si, (o, ln) in enumerate(subsB):
                ps = {}
                for j in range(K1):
                    ps[j] = psB_pool.tile([P, ln], f32, tag="ps", name=f"l3ps{si}_{j}")
                for k in range(K3):
                    h2t = h2_pool.tile([P, 512], f32r, tag="h2s", name=f"h2s{si}_{k}")
                    nc.sync.dma_start(h2t[:, :ln], H2D[k, :, o:o + ln])
                    for j in range(K1):
                        nc.tensor.matmul(ps[j][:], w3s[k][:, j * P:(j + 1) * P],
                                         h2t[:, :ln],
                                         start=(k == 0), stop=(k == K3 - 1))
                for j in range(K1):
                    yv = y_pool.tile([P, 512], f32, tag="yev")
                    nc.scalar.activation(yv[:, :ln], ps[j][:], IDENT,
                                         bias=b3t[:, j:j + 1])
                    nc.sync.dma_start(YT[j, :, o:o + ln], yv[:, :ln])

    nc.compile()
    return nc


LAST_RUN = {}


def kernel(x, Wg, bg, W1, b1, W2, b2, W3, b3):
    B, N, E = x.shape
    xf = np.ascontiguousarray(x.reshape(-1, E), dtype=np.float32)
    T = xf.shape[0]

    # ---- host gating (float64 ordering is stable vs the fp32 reference) ----
    s = xf.astype(np.float64) @ Wg.astype(np.float64) + bg.astype(np.float64)
    ti = np.argsort(-s, axis=1, kind="stable")[:, :TOPK]
    tv = np.take_along_axis(s, ti, axis=1)
    ex = np.exp(tv - tv.max(axis=1, keepdims=True))
    gates = (ex / ex.sum(axis=1, keepdims=True)).astype(np.float32)

    idx_e, gate_e = [], []
    for e in range(NE):
        m0 = ti[:, 0] == e
        m1 = ti[:, 1] == e
        idx_e.append(np.concatenate([np.nonzero(m0)[0], np.nonzero(m1)[0]]))
        gate_e.append(np.concatenate([gates[m0, 0], gates[m1, 1]]))
    counts = [len(i) for i in idx_e]
    C = max(P, -(-max(counts) // P) * P)

    K1, K2, K3 = EMB // P, HID // P, HID2 // P
    MB1, MB2 = HID // 256, HID2 // 256

    in_maps = []
    for e in range(NE):
        xe = np.zeros((C, EMB), np.float32)
        xe[:counts[e]] = xf[idx_e[e]]
        xt = np.ascontiguousarray(xe.T).reshape(K1, P, C)
        w1b = np.ascontiguousarray(
            W1[e].reshape(K1, P, MB1, 256).transpose(2, 0, 1, 3), np.float32)
        w2b = np.ascontiguousarray(
            W2[e].reshape(K2, P, MB2, 256).transpose(2, 0, 1, 3), np.float32)
        w3p = np.ascontiguousarray(W3[e], np.float32).reshape(K3, P, EMB)
        in_maps.append({
            "XT": xt, "W1B": w1b, "W2B": w2b, "W3P": w3p,
            "B1": np.ascontiguousarray(b1[e].reshape(K2, P).T, np.float32),
            "B2": np.ascontiguousarray(b2[e].reshape(K3, P).T, np.float32),
            "B3": np.ascontiguousarray(b3[e].reshape(K1, P).T, np.float32),
        })

    trace = bool(int(os.environ.get("KERNEL_TRACE", "0")))
    if trace:
        _install_ntff_hook()
    nc = _build_program(C)
    res = run_bass_kernel_spmd(nc, in_maps, core_ids=list(range(NE)), trace=trace)
    LAST_RUN["exec_time_ns"] = res.exec_time_ns
    LAST_RUN["capacity"] = C

    out = np.zeros_like(xf)
    for e in range(NE):
        yt = res.results[e]["YT"].reshape(EMB, C)
        ye = yt[:, :counts[e]].T
        out[idx_e[e]] += gate_e[e][:, None] * ye
    return out.reshape(B, N, E)
